# revision 7
# baseline (speedup 1.0000x reference)
"""Multi-head attention (B=4, S=2048, D=1024, H=16) on 8 trn2 NeuronCores.

Sharding: core c -> (batch b = c//2, head-half hh = c%2). Each core computes
attention for 8 heads of one batch element over the full sequence, plus the
partial output projection for its 512 feature rows of w_o. The host sums the
two partial projections per batch element.

Per-core dataflow (all matmuls float32r, fp32 PSUM accumulation):
  xT [1024, 2048]  (host-transposed x[b])
  QT = wq.T @ x.T  -> [512, 2048]   (d-on-partitions layout, head-pair chunks)
  KT = wk.T @ x.T  -> [512, 2048]
  V  = x @ wv      -> [2048, 512]   (token-on-partitions, per-head cols + ones col)
  per head h, key-block kb: S^T[kb, q] = KT_h[:, kb].T @ QT_h  (PSUM)
  E^T = exp(scale * S^T)  (ACT, PSUM -> SBUF f32r)
  out^T_h (PSUM [65, q]) += V_aug[kb, h].T @ E^T[kb]   ; row 64 = softmax sums
  normalize: out^T_h *= broadcast(1/sums)  (K=1 matmul broadcast + DVE mult)
  y_partial[q, e] = sum_c outT[c-chunk].T @ wo[c-chunk]  -> DRAM
"""

import math

import numpy as np

import concourse.bass as bass
import concourse.tile as tile
from concourse import bacc, mybir
from concourse.bass_utils import run_bass_kernel_spmd

F32 = mybir.dt.float32
F32R = mybir.dt.float32r
BF16 = mybir.dt.bfloat16
MM_DT = BF16  # dtype for bulk matmul operands (bf16 streams 2x faster than f32r)

B, S, D = 4, 2048, 1024
H, DH = 16, 64
HPC = 8  # heads per core
DPC = HPC * DH  # 512 feature dims per core
SCALE = 1.0 / math.sqrt(DH + 1e-9)

Exp = mybir.ActivationFunctionType.Exp


def _phase1_projections(nc, tc, xT, wq, wk, wv, QT, KT, V):
    NCH = D // 128
    NPAIR = HPC // 2
    with (
        tc.tile_pool(name="xh", bufs=1) as xh,
        tc.tile_pool(name="wt", bufs=18) as wt,
        tc.tile_pool(name="wvt", bufs=8) as wvt,
        tc.tile_pool(name="ps1", bufs=4, space="PSUM") as ps1,
    ):
        wv_tiles = []
        for c in range(NCH):
            w_t = wvt.tile([128, DPC], MM_DT, tag="wv", name=f"wv_{c}")
            nc.sync.dma_start(out=w_t, in_=wv[c * 128 : (c + 1) * 128, :])
            wv_tiles.append(w_t)

        for th in range(2):  # token halves of 1024
            t0 = th * 1024
            xTs = xh.tile([128, NCH, 1024], MM_DT, tag="xT", name=f"xT_{th}")
            for c in range(NCH):
                nc.sync.dma_start(
                    out=xTs[:, c, :], in_=xT[c * 128 : (c + 1) * 128, t0 : t0 + 1024]
                )

            # QT / KT: features on partitions
            for w_d, dst in ((wq, QT), (wk, KT)):
                for fb in range(NPAIR):
                    w_tiles = []
                    for c in range(NCH):
                        w_t = wt.tile([128, 128], MM_DT, tag="w")
                        nc.sync.dma_start(
                            out=w_t,
                            in_=w_d[c * 128 : (c + 1) * 128, fb * 128 : (fb + 1) * 128],
                        )
                        w_tiles.append(w_t)
                    for tb in range(2):  # 512-token blocks in this half
                        pt = ps1.tile([128, 512], F32, tag="ps1")
                        for c in range(NCH):
                            nc.tensor.matmul(
                                pt,
                                w_tiles[c],
                                xTs[:, c, tb * 512 : (tb + 1) * 512],
                                start=(c == 0),
                                stop=(c == NCH - 1),
                            )
                        nc.scalar.copy(
                            out=dst[:, fb, t0 + tb * 512 : t0 + (tb + 1) * 512],
                            in_=pt,
                        )

            # V: tokens on partitions; psum [128t, 512 = 8 heads x 64]
            for tb in range(8):  # 128-token blocks in this half
                pt = ps1.tile([128, 512], F32, tag="ps1")
                for c in range(NCH):
                    nc.tensor.matmul(
                        pt,
                        xTs[:, c, tb * 128 : (tb + 1) * 128],
                        wv_tiles[c],
                        start=(c == 0),
                        stop=(c == NCH - 1),
                    )
                # scatter heads into V_aug cols 0:64 (stride-65 free layout)
                nc.scalar.copy(
                    out=V[:, th * 8 + tb, :, 0:64],
                    in_=pt.rearrange("p (h d) -> p h d", h=HPC),
                )


def _phase2_attention(nc, tc, QT, KT, V, selC, outT):
    NPAIR = HPC // 2
    NKB = S // 128  # 16 key blocks
    with (
        tc.tile_pool(name="sps", bufs=2, space="PSUM") as sps,
        tc.tile_pool(name="pvs", bufs=2, space="PSUM") as pvs,
        tc.tile_pool(name="ep", bufs=3) as ep,
        tc.tile_pool(name="mp", bufs=1) as mp,
    ):
        for p in range(NPAIR):
            for qh in range(2):  # 1024-wide q halves
                q0 = qh * 1024
                pv_pair = [
                    pvs.tile([65, 1024], F32, tag="pv", name=f"pv_{p}_{qh}_{i}")
                    for i in range(2)
                ]
                for kb in range(NKB):
                    for h2 in range(2):
                        rows = slice(h2 * 64, h2 * 64 + 64)
                        st = sps.tile([128, 1024], F32, tag="s")
                        for qb in range(2):
                            nc.tensor.matmul(
                                st[:, qb * 512 : (qb + 1) * 512],
                                KT[rows, p, kb * 128 : (kb + 1) * 128],
                                QT[rows, p, q0 + qb * 512 : q0 + (qb + 1) * 512],
                                start=True,
                                stop=True,
                            )
                        et = ep.tile([128, 1024], MM_DT, tag="et")
                        nc.scalar.activation(out=et, in_=st, func=Exp, scale=SCALE)
                        pv = pv_pair[h2]
                        for qb in range(2):
                            nc.tensor.matmul(
                                pv[:, qb * 512 : (qb + 1) * 512],
                                V[:, kb, p * 2 + h2, :],
                                et[:, qb * 512 : (qb + 1) * 512],
                                start=(kb == 0),
                                stop=(kb == NKB - 1),
                            )
                # normalize both heads of the pair
                for h2 in range(2):
                    pv = pv_pair[h2]
                    srec = mp.tile([128, 1024], F32R, tag="srec")
                    with nc.allow_low_precision(reason="softmax reciprocal"):
                        nc.vector.reciprocal(out=srec[64:65, :], in_=pv[64:65, :])
                    rb = sps.tile([128, 1024], F32, tag="s")
                    for qb in range(2):
                        nc.tensor.matmul(
                            rb[:, qb * 512 : (qb + 1) * 512],
                            selC[64:65, :],
                            srec[64:65, qb * 512 : (qb + 1) * 512],
                            start=True,
                            stop=True,
                        )
                    rb_sb = mp.tile([128, 1024], F32, tag="rb")
                    nc.vector.tensor_copy(out=rb_sb, in_=rb)
                    if h2 == 0:
                        nc.vector.tensor_mul(
                            out=outT[0:64, p, q0 : q0 + 1024],
                            in0=pv[0:64, :],
                            in1=rb_sb[0:64, :],
                        )
                    else:
                        stg = mp.tile([64, 1024], MM_DT, tag="stg")
                        nc.vector.tensor_mul(
                            out=stg[0:64, :], in0=pv[0:64, :], in1=rb_sb[0:64, :]
                        )
                        nc.sync.dma_start(
                            out=outT[64:128, p, q0 : q0 + 1024], in_=stg[0:64, :]
                        )


def _phase3_output_proj(nc, tc, outT, wo_sb, y):
    NPAIR = HPC // 2
    with (
        tc.tile_pool(name="yps", bufs=4, space="PSUM") as yps,
        tc.tile_pool(name="ysb", bufs=2) as ysb,
    ):
        for qb in range(S // 128):
            y_sb = ysb.tile([128, D], F32, tag="y")
            for eb in range(2):
                yp = yps.tile([128, 512], F32, tag="yp")
                for c in range(NPAIR):
                    nc.tensor.matmul(
                        yp,
                        outT[:, c, qb * 128 : (qb + 1) * 128],
                        wo_sb[:, c, eb * 512 : (eb + 1) * 512],
                        start=(c == 0),
                        stop=(c == NPAIR - 1),
                    )
                nc.scalar.copy(out=y_sb[:, eb * 512 : (eb + 1) * 512], in_=yp)
            nc.sync.dma_start(out=y[qb * 128 : (qb + 1) * 128, :], in_=y_sb)


def build_program():
    nc = bacc.Bacc("TRN2", target_bir_lowering=False, debug=False, num_devices=8)

    xT = nc.dram_tensor("xT", [D, S], MM_DT, kind="ExternalInput")
    wq = nc.dram_tensor("wq", [D, DPC], MM_DT, kind="ExternalInput")
    wk = nc.dram_tensor("wk", [D, DPC], MM_DT, kind="ExternalInput")
    wv = nc.dram_tensor("wv", [D, DPC], MM_DT, kind="ExternalInput")
    wo = nc.dram_tensor("wo", [DPC, D], MM_DT, kind="ExternalInput")
    y = nc.dram_tensor("y", [S, D], F32, kind="ExternalOutput")

    NPAIR = HPC // 2

    with tile.TileContext(nc) as tc:
        with (
            tc.tile_pool(name="qkv", bufs=1) as qkv,
            tc.tile_pool(name="consts", bufs=1) as consts,
        ):
            QT = qkv.tile([128, NPAIR, S], MM_DT, name="QT")
            KT = qkv.tile([128, NPAIR, S], MM_DT, name="KT")
            V = qkv.tile([128, S // 128, HPC, 65], MM_DT, name="V")
            # ones row for broadcast matmul (only row 64 is ever read)
            selC = consts.tile([65, 128], F32R, name="selC")
            nc.vector.memset(selC.bitcast(F32), 1.0)
            # ones column of V_aug (cols 0:64 overwritten by projection copies)
            nc.vector.memset(V, 1.0)

            _phase1_projections(nc, tc, xT, wq, wk, wv, QT, KT, V)

            with tc.tile_pool(name="big", bufs=1) as bigpool:
                outT = bigpool.tile([128, NPAIR, S], MM_DT, name="outT")
                wo_sb = bigpool.tile([128, NPAIR, D], MM_DT, name="wo_sb")
                nc.sync.dma_start(out=wo_sb, in_=wo.rearrange("(c p) e -> p c e", p=128))

                _phase2_attention(nc, tc, QT, KT, V, selC, outT)
                _phase3_output_proj(nc, tc, outT, wo_sb, y)

    nc.compile()
    return nc


_program_cache = {}


def _get_program():
    if "nc" not in _program_cache:
        _program_cache["nc"] = build_program()
    return _program_cache["nc"]


def build_in_maps(x, w_qkv, w_o):
    import ml_dtypes

    np_dt = mybir.dt.np(MM_DT)
    in_maps = []
    for c in range(8):
        b, hh = c // 2, c % 2
        f0 = hh * DPC
        in_maps.append(
            {
                "xT": np.ascontiguousarray(x[b].T.astype(np_dt)),
                "wq": np.ascontiguousarray(w_qkv[:, f0 : f0 + DPC].astype(np_dt)),
                "wk": np.ascontiguousarray(w_qkv[:, D + f0 : D + f0 + DPC].astype(np_dt)),
                "wv": np.ascontiguousarray(w_qkv[:, 2 * D + f0 : 2 * D + f0 + DPC].astype(np_dt)),
                "wo": np.ascontiguousarray(w_o[f0 : f0 + DPC, :].astype(np_dt)),
            }
        )
    return in_maps


def kernel(x: np.ndarray, w_qkv: np.ndarray, w_o: np.ndarray) -> np.ndarray:
    x = np.ascontiguousarray(np.asarray(x, dtype=np.float32))
    w_qkv = np.ascontiguousarray(np.asarray(w_qkv, dtype=np.float32))
    w_o = np.ascontiguousarray(np.asarray(w_o, dtype=np.float32))
    assert x.shape == (B, S, D) and w_qkv.shape == (D, 3 * D) and w_o.shape == (D, D)
    nc = _get_program()
    res = run_bass_kernel_spmd(nc, build_in_maps(x, w_qkv, w_o), core_ids=list(range(8)))
    out = np.empty((B, S, D), dtype=np.float32)
    for b in range(B):
        out[b] = res.results[2 * b]["y"] + res.results[2 * b + 1]["y"]
    return out


# revision 8
# speedup vs baseline: 1.0011x; 1.0011x over previous
"""Multi-head attention (B=4, S=2048, D=1024, H=16) on 8 trn2 NeuronCores.

Sharding: core c -> (batch b = c//2, head-half hh = c%2). Each core computes
attention for 8 heads of one batch element over the full sequence, plus the
partial output projection for its 512 feature rows of w_o. The host sums the
two partial projections per batch element.

Per-core dataflow (all matmuls float32r, fp32 PSUM accumulation):
  xT [1024, 2048]  (host-transposed x[b])
  QT = wq.T @ x.T  -> [512, 2048]   (d-on-partitions layout, head-pair chunks)
  KT = wk.T @ x.T  -> [512, 2048]
  V  = x @ wv      -> [2048, 512]   (token-on-partitions, per-head cols + ones col)
  per head h, key-block kb: S^T[kb, q] = KT_h[:, kb].T @ QT_h  (PSUM)
  E^T = exp(scale * S^T)  (ACT, PSUM -> SBUF f32r)
  out^T_h (PSUM [65, q]) += V_aug[kb, h].T @ E^T[kb]   ; row 64 = softmax sums
  normalize: out^T_h *= broadcast(1/sums)  (K=1 matmul broadcast + DVE mult)
  y_partial[q, e] = sum_c outT[c-chunk].T @ wo[c-chunk]  -> DRAM
"""

import math

import numpy as np

import concourse.bass as bass
import concourse.tile as tile
from concourse import bacc, mybir
from concourse.bass_utils import run_bass_kernel_spmd

F32 = mybir.dt.float32
F32R = mybir.dt.float32r
BF16 = mybir.dt.bfloat16
MM_DT = BF16  # dtype for bulk matmul operands (bf16 streams 2x faster than f32r)

B, S, D = 4, 2048, 1024
H, DH = 16, 64
HPC = 8  # heads per core
DPC = HPC * DH  # 512 feature dims per core
SCALE = 1.0 / math.sqrt(DH + 1e-9)

Exp = mybir.ActivationFunctionType.Exp


def _phase1_projections(nc, tc, xT, wq, wk, wv, QT, KT, V):
    NCH = D // 128
    NPAIR = HPC // 2
    with (
        tc.tile_pool(name="xh", bufs=1) as xh,
        tc.tile_pool(name="wt", bufs=18) as wt,
        tc.tile_pool(name="wvt", bufs=8) as wvt,
        tc.tile_pool(name="ps1", bufs=4, space="PSUM") as ps1,
    ):
        wv_tiles = []
        for c in range(NCH):
            w_t = wvt.tile([128, DPC], MM_DT, tag="wv", name=f"wv_{c}")
            nc.sync.dma_start(out=w_t, in_=wv[c * 128 : (c + 1) * 128, :])
            wv_tiles.append(w_t)

        for th in range(2):  # token halves of 1024
            t0 = th * 1024
            xTs = xh.tile([128, NCH, 1024], MM_DT, tag="xT", name=f"xT_{th}")
            for c in range(NCH):
                nc.sync.dma_start(
                    out=xTs[:, c, :], in_=xT[c * 128 : (c + 1) * 128, t0 : t0 + 1024]
                )

            # QT / KT: features on partitions
            for w_d, dst in ((wq, QT), (wk, KT)):
                for fb in range(NPAIR):
                    w_tiles = []
                    for c in range(NCH):
                        w_t = wt.tile([128, 128], MM_DT, tag="w")
                        nc.sync.dma_start(
                            out=w_t,
                            in_=w_d[c * 128 : (c + 1) * 128, fb * 128 : (fb + 1) * 128],
                        )
                        w_tiles.append(w_t)
                    for tb in range(2):  # 512-token blocks in this half
                        pt = ps1.tile([128, 512], F32, tag="ps1")
                        for c in range(NCH):
                            nc.tensor.matmul(
                                pt,
                                w_tiles[c],
                                xTs[:, c, tb * 512 : (tb + 1) * 512],
                                start=(c == 0),
                                stop=(c == NCH - 1),
                            )
                        nc.scalar.copy(
                            out=dst[:, fb, t0 + tb * 512 : t0 + (tb + 1) * 512],
                            in_=pt,
                        )

            # V: tokens on partitions; psum [128t, 512 = 8 heads x 64]
            for tb in range(8):  # 128-token blocks in this half
                pt = ps1.tile([128, 512], F32, tag="ps1")
                for c in range(NCH):
                    nc.tensor.matmul(
                        pt,
                        xTs[:, c, tb * 128 : (tb + 1) * 128],
                        wv_tiles[c],
                        start=(c == 0),
                        stop=(c == NCH - 1),
                    )
                # scatter heads into V_aug cols 0:64 (stride-65 free layout)
                nc.scalar.copy(
                    out=V[:, th * 8 + tb, :, 0:64],
                    in_=pt.rearrange("p (h d) -> p h d", h=HPC),
                )


def _phase2_attention(nc, tc, QT, KT, V, selC, outT):
    NPAIR = HPC // 2
    NKB = S // 128  # 16 key blocks
    with (
        tc.tile_pool(name="sps", bufs=2, space="PSUM") as sps,
        tc.tile_pool(name="pvs", bufs=2, space="PSUM") as pvs,
        tc.tile_pool(name="ep", bufs=2) as ep,
        tc.tile_pool(name="mp", bufs=2) as mp,
    ):
        for p in range(NPAIR):
            for qh in range(2):  # 1024-wide q halves
                q0 = qh * 1024
                for h2 in range(2):
                    rows = slice(h2 * 64, h2 * 64 + 64)
                    # stage A: scores + exp for the whole head into SBUF
                    eth = ep.tile([128, NKB, 1024], MM_DT, tag="eth")
                    for kb in range(NKB):
                        st = sps.tile([128, 1024], F32, tag="s")
                        for qb in range(2):
                            nc.tensor.matmul(
                                st[:, qb * 512 : (qb + 1) * 512],
                                KT[rows, p, kb * 128 : (kb + 1) * 128],
                                QT[rows, p, q0 + qb * 512 : q0 + (qb + 1) * 512],
                                start=True,
                                stop=True,
                            )
                        nc.scalar.activation(
                            out=eth[:, kb, :], in_=st, func=Exp, scale=SCALE
                        )
                    # stage B: PV burst (interleaves with next head's stage A on PE)
                    pv = pvs.tile([65, 1024], F32, tag="pv", name=f"pv_{p}_{qh}_{h2}")
                    for kb in range(NKB):
                        for qb in range(2):
                            nc.tensor.matmul(
                                pv[:, qb * 512 : (qb + 1) * 512],
                                V[:, kb, p * 2 + h2, :],
                                eth[:, kb, qb * 512 : (qb + 1) * 512],
                                start=(kb == 0),
                                stop=(kb == NKB - 1),
                            )
                    # normalize
                    srec = mp.tile([128, 1024], F32R, tag="srec")
                    with nc.allow_low_precision(reason="softmax reciprocal"):
                        nc.vector.reciprocal(out=srec[64:65, :], in_=pv[64:65, :])
                    rb = sps.tile([128, 1024], F32, tag="s")
                    for qb in range(2):
                        nc.tensor.matmul(
                            rb[:, qb * 512 : (qb + 1) * 512],
                            selC[64:65, :],
                            srec[64:65, qb * 512 : (qb + 1) * 512],
                            start=True,
                            stop=True,
                        )
                    rb_sb = mp.tile([128, 1024], F32, tag="rb")
                    nc.vector.tensor_copy(out=rb_sb, in_=rb)
                    if h2 == 0:
                        nc.vector.tensor_mul(
                            out=outT[0:64, p, q0 : q0 + 1024],
                            in0=pv[0:64, :],
                            in1=rb_sb[0:64, :],
                        )
                    else:
                        stg = mp.tile([64, 1024], MM_DT, tag="stg")
                        nc.vector.tensor_mul(
                            out=stg[0:64, :], in0=pv[0:64, :], in1=rb_sb[0:64, :]
                        )
                        nc.sync.dma_start(
                            out=outT[64:128, p, q0 : q0 + 1024], in_=stg[0:64, :]
                        )


def _phase3_output_proj(nc, tc, outT, wo_sb, y):
    NPAIR = HPC // 2
    with (
        tc.tile_pool(name="yps", bufs=4, space="PSUM") as yps,
        tc.tile_pool(name="ysb", bufs=2) as ysb,
    ):
        for qb in range(S // 128):
            y_sb = ysb.tile([128, D], F32, tag="y")
            for eb in range(2):
                yp = yps.tile([128, 512], F32, tag="yp")
                for c in range(NPAIR):
                    nc.tensor.matmul(
                        yp,
                        outT[:, c, qb * 128 : (qb + 1) * 128],
                        wo_sb[:, c, eb * 512 : (eb + 1) * 512],
                        start=(c == 0),
                        stop=(c == NPAIR - 1),
                    )
                nc.scalar.copy(out=y_sb[:, eb * 512 : (eb + 1) * 512], in_=yp)
            nc.sync.dma_start(out=y[qb * 128 : (qb + 1) * 128, :], in_=y_sb)


def build_program():
    nc = bacc.Bacc("TRN2", target_bir_lowering=False, debug=False, num_devices=8)

    xT = nc.dram_tensor("xT", [D, S], MM_DT, kind="ExternalInput")
    wq = nc.dram_tensor("wq", [D, DPC], MM_DT, kind="ExternalInput")
    wk = nc.dram_tensor("wk", [D, DPC], MM_DT, kind="ExternalInput")
    wv = nc.dram_tensor("wv", [D, DPC], MM_DT, kind="ExternalInput")
    wo = nc.dram_tensor("wo", [DPC, D], MM_DT, kind="ExternalInput")
    y = nc.dram_tensor("y", [S, D], F32, kind="ExternalOutput")

    NPAIR = HPC // 2

    with tile.TileContext(nc) as tc:
        with (
            tc.tile_pool(name="qkv", bufs=1) as qkv,
            tc.tile_pool(name="consts", bufs=1) as consts,
        ):
            QT = qkv.tile([128, NPAIR, S], MM_DT, name="QT")
            KT = qkv.tile([128, NPAIR, S], MM_DT, name="KT")
            V = qkv.tile([128, S // 128, HPC, 65], MM_DT, name="V")
            # ones row for broadcast matmul (only row 64 is ever read)
            selC = consts.tile([65, 128], F32R, name="selC")
            nc.vector.memset(selC.bitcast(F32), 1.0)
            # ones column of V_aug (cols 0:64 overwritten by projection copies)
            nc.vector.memset(V, 1.0)

            _phase1_projections(nc, tc, xT, wq, wk, wv, QT, KT, V)

            with tc.tile_pool(name="big", bufs=1) as bigpool:
                outT = bigpool.tile([128, NPAIR, S], MM_DT, name="outT")
                wo_sb = bigpool.tile([128, NPAIR, D], MM_DT, name="wo_sb")
                nc.sync.dma_start(out=wo_sb, in_=wo.rearrange("(c p) e -> p c e", p=128))

                _phase2_attention(nc, tc, QT, KT, V, selC, outT)
                _phase3_output_proj(nc, tc, outT, wo_sb, y)

    nc.compile()
    return nc


_program_cache = {}


def _get_program():
    if "nc" not in _program_cache:
        _program_cache["nc"] = build_program()
    return _program_cache["nc"]


def build_in_maps(x, w_qkv, w_o):
    import ml_dtypes

    np_dt = mybir.dt.np(MM_DT)
    in_maps = []
    for c in range(8):
        b, hh = c // 2, c % 2
        f0 = hh * DPC
        in_maps.append(
            {
                "xT": np.ascontiguousarray(x[b].T.astype(np_dt)),
                "wq": np.ascontiguousarray(w_qkv[:, f0 : f0 + DPC].astype(np_dt)),
                "wk": np.ascontiguousarray(w_qkv[:, D + f0 : D + f0 + DPC].astype(np_dt)),
                "wv": np.ascontiguousarray(w_qkv[:, 2 * D + f0 : 2 * D + f0 + DPC].astype(np_dt)),
                "wo": np.ascontiguousarray(w_o[f0 : f0 + DPC, :].astype(np_dt)),
            }
        )
    return in_maps


def kernel(x: np.ndarray, w_qkv: np.ndarray, w_o: np.ndarray) -> np.ndarray:
    x = np.ascontiguousarray(np.asarray(x, dtype=np.float32))
    w_qkv = np.ascontiguousarray(np.asarray(w_qkv, dtype=np.float32))
    w_o = np.ascontiguousarray(np.asarray(w_o, dtype=np.float32))
    assert x.shape == (B, S, D) and w_qkv.shape == (D, 3 * D) and w_o.shape == (D, D)
    nc = _get_program()
    res = run_bass_kernel_spmd(nc, build_in_maps(x, w_qkv, w_o), core_ids=list(range(8)))
    out = np.empty((B, S, D), dtype=np.float32)
    for b in range(B):
        out[b] = res.results[2 * b]["y"] + res.results[2 * b + 1]["y"]
    return out
